# revision 8
# baseline (speedup 1.0000x reference)
"""Causal self-attention Trainium2 Bass kernel.

Sharding: 8-way head tensor-parallelism for QKV projections + attention
(2 heads per core, full batch).  The output projection is re-sharded via
FOUR chunked AllToAlls (one per pair of 512-token blocks, 128-token
stripes per core) so the exchange overlaps attention compute; each core
then projects 4x128 = 512 token rows and the host re-interleaves.

Fused software pipeline per 512-token block g (b = g//4, a = g%4):
  norm(g-1) [ACT ln -> exp(-x) gives 1/den, K=1 matmul broadcast,
  DVE multiply] -> collective chunk (every 2nd block) -> QK projection
  for block g -> output projection of the previous chunk -> V
  projection + transpose -> attention jc-loop (S^T = K^T.T @ Q^T,
  exp on ACT, PV accumulate; PV emission delayed one jc so the PE
  never waits on the ACT exp).

This keeps the PE array continuously busy (avoids the 2x mid-p-state
clock penalty) and leaves only the last collective + 128-row output
projection exposed at the end.

All matmul operands are bf16 (fp32 PSUM accumulation); measured
end-to-end relative error vs the fp32 reference is ~3e-3.

PSUM budget (8 banks): pool A [128,2,512]f32 x2 bufs (4 banks) shared
by QK-proj / V+transposes / ST tiles / rcp-broadcast / yproj, and pool
pv [65,2,512]f32 x2 bufs (4 banks) for the PV accumulators.
"""

import math
import os

import numpy as np

os.environ.setdefault("JAX_COMPILATION_CACHE_DIR", "/tmp/jax_cache")

D_MODEL = 1024
NUM_HEADS = 16
D_K = 64
B = 2
T = 2048
TT = B * T          # 4096 flattened tokens
NCORES = 8
HL = NUM_HEADS // NCORES   # heads per core = 2
DO = D_MODEL // 128        # 8 contraction chunks
NB = TT // 512             # 8 blocks (b, a)
NI = T // 512              # 4 query chunks per batch
NJ = T // 128              # 16 key chunks per batch
NCH = NB // 2              # 4 AllToAll chunks (2 blocks each)
SH = TT // NCORES          # 512 output rows per core

_cache = {}


def _install_ntff_hook():
    """The agent image's antenv lacks axon_hooks; replicate what
    trn_agent_boot would register so trace=True can capture NTFFs."""
    import sys
    import types

    try:
        from antenv import axon_hooks  # noqa: F401
        return True
    except ImportError:
        pass
    try:
        import antenv
        from trn_agent_boot.trn_boot import _ntff_profile_via_ctypes

        mod = types.ModuleType("antenv.axon_hooks")
        holder = [None]
        mod.set_axon_ntff_profile_hook = lambda h: holder.__setitem__(0, h)
        mod.get_axon_ntff_profile_hook = lambda: holder[0]
        sys.modules["antenv.axon_hooks"] = mod
        antenv.axon_hooks = mod
        mod.set_axon_ntff_profile_hook(
            _ntff_profile_via_ctypes("/opt/axon/libaxon_pjrt.so")
        )
        return True
    except Exception:
        return False


def _build_module(mode, blocks=None, n_mtiles=1):
    """Build + compile the Bass module.

    mode: "causal" (tril mask), "ones" (no masking), "generic"
    blocks: for generic mode, blocks[jc][a] = 0 skip / 1 full / (2, idx) mixed
    """
    from contextlib import ExitStack

    import concourse.mybir as mybir
    import concourse.tile as tile
    from concourse import bacc

    F32 = mybir.dt.float32
    BF16 = mybir.dt.bfloat16
    AF = mybir.ActivationFunctionType

    nc = bacc.Bacc(
        "TRN2",
        target_bir_lowering=False,
        debug=False,
        enable_asserts=False,
        num_devices=NCORES,
    )

    xT = nc.dram_tensor("xT", [128, DO, TT], BF16, kind="ExternalInput").ap()
    wq = nc.dram_tensor("wq", [128, DO, 128], BF16, kind="ExternalInput").ap()
    wk = nc.dram_tensor("wk", [128, DO, 128], BF16, kind="ExternalInput").ap()
    wv = nc.dram_tensor("wv", [128, DO, 128], BF16, kind="ExternalInput").ap()
    wo = nc.dram_tensor("wo", [128, DO, 1024], BF16, kind="ExternalInput").ap()
    bqin = nc.dram_tensor("bq", [128, 1], F32, kind="ExternalInput").ap()
    bkin = nc.dram_tensor("bk", [128, 1], F32, kind="ExternalInput").ap()
    bvin = nc.dram_tensor("bv", [128, 1], F32, kind="ExternalInput").ap()
    boin = nc.dram_tensor("bo", [1, 1024], F32, kind="ExternalInput").ap()
    tri_in = nc.dram_tensor("tri", [128, 128], BF16, kind="ExternalInput").ap()
    id_in = nc.dram_tensor("identf", [128, 128], F32, kind="ExternalInput").ap()
    if mode == "generic":
        mtiles = nc.dram_tensor(
            "mtiles", [n_mtiles, 128, 512], BF16, kind="ExternalInput"
        ).ap()
    y = nc.dram_tensor("y", [SH, 1024], F32, kind="ExternalOutput").ap()

    def jcs_of(b, a):
        if mode == "causal":
            return list(range(4 * a + 4))
        if mode == "ones":
            return list(range(NJ))
        return [jc for jc in range(NJ) if blocks[jc][a] != 0]

    with tile.TileContext(nc) as tc, ExitStack() as ctx:
        pers = ctx.enter_context(tc.tile_pool(name="pers", bufs=1))
        # PSUM: pool A (QK proj, V+transposes, ST pairs, rcp bcast, yproj)
        # 2 bufs x [128,2,512] f32 = 4 banks; pool pv 2 bufs = 4 banks.
        pA = ctx.enter_context(tc.tile_pool(name="pA", bufs=2, space="PSUM"))
        pV = ctx.enter_context(tc.tile_pool(name="pV", bufs=2, space="PSUM"))

        # ---- persistent SBUF (weights etc.; wo deferred to block 1) ----
        wq_sb = pers.tile([128, DO, 128], BF16, name="wq_sb")
        nc.sync.dma_start(wq_sb[:], wq[:])
        wk_sb = pers.tile([128, DO, 128], BF16, name="wk_sb")
        nc.sync.dma_start(wk_sb[:], wk[:])
        wv_sb = pers.tile([128, DO, 128], BF16, name="wv_sb")
        nc.sync.dma_start(wv_sb[:], wv[:])
        bq_sb = pers.tile([128, 1], F32, name="bq_sb")
        nc.sync.dma_start(bq_sb[:], bqin[:])
        bk_sb = pers.tile([128, 1], F32, name="bk_sb")
        nc.sync.dma_start(bk_sb[:], bkin[:])
        bv_sb = pers.tile([128, 1], F32, name="bv_sb")
        nc.sync.dma_start(bv_sb[:], bvin[:])
        bo_sb = pers.tile([1, 1024], F32, name="bo_sb")
        nc.sync.dma_start(bo_sb[:], boin[:])
        tri_full = pers.tile([128, 128], BF16, name="tri_full")
        nc.sync.dma_start(tri_full[:], tri_in[:])
        tri_sb = tri_full[:, 0:128]
        ident_t = pers.tile([128, 128], F32, name="ident_t")
        nc.sync.dma_start(ident_t[:], id_in[:])
        ident = ident_t[:]
        wo_sb = pers.tile([128, DO, 1024], BF16, name="wo_sb")

        ones_bf = pers.tile([128, 128], BF16, name="ones_bf")
        nc.vector.memset(ones_bf[:], 1.0)
        ones_f32 = pers.tile([1, 128], F32, name="ones_f32")
        nc.vector.memset(ones_f32[:], 1.0)

        qt = pers.tile([128, TT], BF16, name="qt")
        kt = pers.tile([128, TT], BF16, name="kt")
        vsb = pers.tile([128, B * NJ, HL, 80], BF16, name="vsb")
        nc.vector.tensor_copy(
            vsb[:, :, :, 64],
            ones_bf[:, 0 : B * NJ * HL].rearrange("p (a b) -> p a b", a=B * NJ),
        )

        # broadcast bo across partitions once (fp32, one-time)
        bob = pers.tile([128, 1024], F32, name="bob")
        bps = pA.tile([128, 2, 512], F32, name="bps", tag="A")
        for i in range(2):
            nc.tensor.matmul(
                bps[:, i, :],
                ones_f32[:, :],
                bo_sb[:, 512 * i : 512 * (i + 1)],
                start=True,
                stop=True,
            )
        nc.vector.tensor_copy(bob[:], bps.rearrange("p a b -> p (a b)"))

        # DRAM staging for the 4 chunked AllToAlls
        dramp = ctx.enter_context(tc.tile_pool(name="dramp", bufs=1, space="DRAM"))
        a2a_in = [
            dramp.tile([NCORES, 128, 128], BF16, name=f"a2a_in{c}")
            for c in range(NCH)
        ]
        a2a_out = [
            dramp.tile([NCORES, 128, 128], BF16, name=f"a2a_out{c}")
            for c in range(NCH)
        ]

        xtp = ctx.enter_context(tc.tile_pool(name="xtp", bufs=2))
        vtp = ctx.enter_context(tc.tile_pool(name="vtp", bufs=2))
        sxp = ctx.enter_context(tc.tile_pool(name="sxp", bufs=4))
        normp = ctx.enter_context(tc.tile_pool(name="normp", bufs=2))
        obp = ctx.enter_context(tc.tile_pool(name="obp", bufs=2))
        oap = ctx.enter_context(tc.tile_pool(name="oap", bufs=2))
        yp = ctx.enter_context(tc.tile_pool(name="yp", bufs=2))
        mtp = ctx.enter_context(tc.tile_pool(name="mtp", bufs=2))

        xt_tiles = [None] * NB

        def prefetch_x(g):
            xt_tiles[g] = xtp.tile([128, DO, 512], BF16, name=f"xt{g}", tag="xt")
            nc.sync.dma_start(xt_tiles[g][:], xT[:, :, 512 * g : 512 * (g + 1)])

        def emit_norm(pend):
            """Normalize block g's PV accumulator and stage it for the
            AllToAll: 1/den = exp(-ln(den)) on ACT (same table as the
            softmax exp), K=1 matmul broadcast, DVE multiply."""
            g, pv = pend
            ii0 = 512 * g
            if pv is None:  # generic mode, fully-masked block
                ob = obp.tile([64, HL, 512], BF16, name=f"ob{g}", tag="ob")
                nc.vector.memset(ob[:], 0.0)
            else:
                lnr = normp.tile([128, HL, 512], F32, name=f"ln{g}", tag="ln")
                nc.scalar.activation(lnr[64:65, :, :], pv[64:65, :, :], AF.Ln)
                rcpr = normp.tile([128, HL, 512], BF16, name=f"rcp{g}", tag="rcp")
                nc.scalar.activation(
                    rcpr[64:65, :, :], lnr[64:65, :, :], AF.Exp, scale=-1.0
                )
                rbt = pA.tile([128, 2, 512], F32, name=f"rb{g}", tag="A")
                for h in range(HL):
                    nc.tensor.matmul(
                        rbt[0:64, h, :],
                        ones_bf[64:65, 0:64],
                        rcpr[64:65, h, :],
                        start=True,
                        stop=True,
                        tile_position=(64, 0),
                    )
                # DVE may read only one input from PSUM: stage pv via ACT
                pvc = normp.tile([128, HL, 512], BF16, name=f"pvc{g}", tag="pvc")
                nc.scalar.activation(pvc[0:64, :, :], pv[0:64, :, :], AF.Copy)
                ob = obp.tile([64, HL, 512], BF16, name=f"ob{g}", tag="ob")
                nc.vector.tensor_mul(ob[:], pvc[0:64, :, :], rbt[0:64, :, :])
            # stage into the a2a input: chunk c = g//2; block supplies
            # destination stripes 4*(g%2) .. 4*(g%2)+3 (128 tokens each).
            c, q = g // 2, g % 2
            for d4 in range(4):
                d = 4 * q + d4
                for h in range(HL):
                    nc.sync.dma_start(
                        a2a_in[c][d, 64 * h : 64 * (h + 1), :],
                        ob[:, h, 128 * d4 : 128 * (d4 + 1)],
                    )

        def emit_collective(c):
            nc.gpsimd.collective_compute(
                "AllToAll",
                mybir.AluOpType.bypass,
                replica_groups=[list(range(NCORES))],
                ins=[a2a_in[c].opt()],
                outs=[a2a_out[c].opt()],
            )

        def emit_yproj(c):
            """Output projection for this core's 128 tokens of chunk c."""
            oa = oap.tile([128, NCORES, 128], BF16, name=f"oa{c}", tag="oa")
            nc.sync.dma_start(oa[:], a2a_out[c].rearrange("r p t -> p r t"))
            yps = pA.tile([128, 2, 512], F32, name=f"yps{c}", tag="A")
            for oc in range(2):
                for do in range(DO):
                    nc.tensor.matmul(
                        yps[:, oc, :],
                        oa[:, do, :],
                        wo_sb[:, do, 512 * oc : 512 * (oc + 1)],
                        start=(do == 0),
                        stop=(do == DO - 1),
                    )
            yt = yp.tile([128, 1024], F32, name=f"yt{c}", tag="y")
            nc.vector.tensor_add(yt[:], yps.rearrange("p a b -> p (a b)"), bob[:])
            nc.sync.dma_start(y[128 * c : 128 * (c + 1), :], yt[:])

        prefetch_x(0)
        pend = None  # (g, pv tile) awaiting normalization
        for g in range(NB):
            b, a = g // 4, g % 4
            ii0 = 512 * g
            if g + 1 < NB:
                prefetch_x(g + 1)

            # ---- deferred normalization of the previous block ----
            if pend is not None:
                emit_norm(pend)
                pend = None
            if g >= 2 and g % 2 == 0:
                emit_collective(g // 2 - 1)

            # ---- QK projection for block g ----
            xt_t = xt_tiles[g]
            qk = pA.tile([128, 2, 512], F32, name=f"qk{g}", tag="A")
            for col, w_sb in ((0, wq_sb), (1, wk_sb)):
                for do in range(DO):
                    nc.tensor.matmul(
                        qk[:, col, :],
                        w_sb[:, do, :],
                        xt_t[:, do, :],
                        start=(do == 0),
                        stop=(do == DO - 1),
                    )
            nc.vector.tensor_scalar_add(
                qt[:, ii0 : ii0 + 512], qk[:, 0, :], bq_sb[:]
            )
            nc.vector.tensor_scalar_add(
                kt[:, ii0 : ii0 + 512], qk[:, 1, :], bk_sb[:]
            )

            if g == 1:
                nc.sync.dma_start(wo_sb[:], wo[:])
            if g >= 2 and g % 2 == 0:
                emit_yproj(g // 2 - 1)

            # ---- V projection + transpose for block g ----
            vtr = pA.tile([128, 2, 512], F32, name=f"vtr{g}", tag="A")
            for do in range(DO):
                nc.tensor.matmul(
                    vtr[:, 0, :],
                    wv_sb[:, do, :],
                    xt_t[:, do, :],
                    start=(do == 0),
                    stop=(do == DO - 1),
                )
            vt_t = vtp.tile([128, 512], F32, name=f"vt{g}", tag="vt")
            nc.vector.tensor_scalar_add(vt_t[:], vtr[:, 0, :], bv_sb[:])
            tps = vtr[:, 1, :].rearrange("p (k t) -> p k t", k=4)
            for k in range(4):
                nc.tensor.transpose(
                    tps[:, k, :], vt_t[:, 128 * k : 128 * (k + 1)], ident
                )
            nc.vector.tensor_copy(
                vsb[:, 4 * g : 4 * (g + 1), :, 0:64],
                tps.rearrange("p k (h c) -> p k h c", h=HL),
            )

            # ---- attention for block g ----
            jcs = jcs_of(b, a)
            if not jcs:
                pend = (g, None)
                continue
            pv = pV.tile([65, 2, 512], F32, name=f"pv{g}", tag="pv")
            pvs = [pv[:, h, :] for h in range(HL)]
            prev = None  # (jc, s, w, first) awaiting PV emission
            for idx, jc in enumerate(jcs):
                j0 = b * T + 128 * jc
                diag = mode == "causal" and jc >= 4 * a
                s = 128 * (jc - 4 * a) if diag else 0
                w = 512 - s
                st = pA.tile([128, 2, 512], F32, name=f"st{g}_{jc}", tag="A")
                for h in range(HL):
                    nc.tensor.matmul(
                        st[:, h, 0:w],
                        kt[64 * h : 64 * (h + 1), j0 : j0 + 128],
                        qt[64 * h : 64 * (h + 1), ii0 + s : ii0 + 512],
                        start=True,
                        stop=True,
                        tile_position=(64 * h, 0),
                    )
                ex = sxp.tile([128, 2, 512], BF16, name=f"ex{g}_{jc}", tag="ex")
                nc.scalar.activation(ex[:, :, 0:w], st[:, :, 0:w], AF.Exp)
                if diag:
                    for h in range(HL):
                        nc.vector.tensor_mul(
                            ex[:, h, 0:128], ex[:, h, 0:128], tri_sb
                        )
                if mode == "generic" and blocks[jc][a] != 1:
                    mt = mtp.tile([128, 512], BF16, name=f"mt{g}_{jc}", tag="mt")
                    nc.sync.dma_start(mt[:], mtiles[blocks[jc][a][1]])
                    for h in range(HL):
                        nc.vector.tensor_mul(ex[:, h, :], ex[:, h, :], mt[:])
                # delayed PV emission: the PE never stalls on this jc's exp
                if prev is not None:
                    pjc, ps, pw, pfirst, pex = prev
                    for h in range(HL):
                        nc.tensor.matmul(
                            pvs[h][:, ps : ps + pw],
                            vsb[:, b * NJ + pjc, h, 0:65],
                            pex[:, h, 0:pw],
                            start=pfirst,
                            stop=False,
                        )
                prev = (jc, s, w, idx == 0, ex)
            pjc, ps, pw, pfirst, pex = prev
            for h in range(HL):
                nc.tensor.matmul(
                    pvs[h][:, ps : ps + pw],
                    vsb[:, b * NJ + pjc, h, 0:65],
                    pex[:, h, 0:pw],
                    start=pfirst,
                    stop=True,
                )
            pend = (g, pv)

        # ---- tail: last norm, last collective, last yproj ----
        emit_norm(pend)
        emit_collective(NCH - 1)
        emit_yproj(NCH - 1)

    nc.compile()
    return nc


def _detect_mode(mask):
    m2 = np.asarray(mask).reshape(T, T)
    if np.array_equal(m2, np.tril(np.ones((T, T), m2.dtype))):
        return "causal", None, None
    if np.all(m2 != 0):
        return "ones", None, None
    # generic: classify [jc, a] blocks of mask^T
    mT = (m2 != 0).T.astype(np.float32)  # [j, i]
    blocks = [[0] * NI for _ in range(NJ)]
    tiles = []
    seen = {}
    for jc in range(NJ):
        for a in range(NI):
            sub = mT[128 * jc : 128 * (jc + 1), 512 * a : 512 * (a + 1)]
            if not sub.any():
                blocks[jc][a] = 0
            elif sub.all():
                blocks[jc][a] = 1
            else:
                key = sub.tobytes()
                if key not in seen:
                    seen[key] = len(tiles)
                    tiles.append(sub.copy())
                blocks[jc][a] = (2, seen[key])
    mt = np.stack(tiles) if tiles else np.zeros((1, 128, 512), np.float32)
    return "generic", blocks, mt


def _bf16(a):
    import ml_dtypes

    return np.ascontiguousarray(a, dtype=np.float32).astype(ml_dtypes.bfloat16)


def _rearr_w(w):
    # [D, M] -> [128, DO, M] as (d_inner, d_outer, m), bf16
    m = w.shape[1]
    return _bf16(
        np.ascontiguousarray(w, dtype=np.float32)
        .reshape(DO, 128, m)
        .transpose(1, 0, 2)
    )


def kernel(x, mask, Wq, bq, Wk, bk, Wv, bv, Wo, bo, trace=False):
    from concourse import bass_utils

    x = np.asarray(x, dtype=np.float32)
    Wq = np.asarray(Wq, dtype=np.float32)
    Wk = np.asarray(Wk, dtype=np.float32)
    Wv = np.asarray(Wv, dtype=np.float32)
    Wo = np.asarray(Wo, dtype=np.float32)
    bq = np.asarray(bq, dtype=np.float32)
    bk = np.asarray(bk, dtype=np.float32)
    bv = np.asarray(bv, dtype=np.float32)
    bo = np.asarray(bo, dtype=np.float32)

    mode, blocks, mt = _detect_mode(mask)
    cache_key = (mode, None if blocks is None else str(blocks))
    if cache_key not in _cache:
        _cache[cache_key] = _build_module(
            mode, blocks, 1 if mt is None else mt.shape[0]
        )
    nc = _cache[cache_key]

    scale = 1.0 / math.sqrt(D_K)
    xT_arr = _bf16(x.reshape(TT, D_MODEL).T.reshape(DO, 128, TT).transpose(1, 0, 2))
    wo_arr = _rearr_w(Wo)
    bo_arr = np.ascontiguousarray(bo.reshape(1, 1024))
    tri_arr = _bf16(np.triu(np.ones((128, 128), np.float32)))
    id_arr = np.eye(128, dtype=np.float32)

    in_maps = []
    for c in range(NCORES):
        sl = slice(128 * c, 128 * (c + 1))
        m = {
            "xT": xT_arr,
            "wq": _rearr_w(Wq[:, sl] * scale),
            "wk": _rearr_w(Wk[:, sl]),
            "wv": _rearr_w(Wv[:, sl]),
            "wo": wo_arr,
            "bq": np.ascontiguousarray((bq[sl] * scale).reshape(128, 1)),
            "bk": np.ascontiguousarray(bk[sl].reshape(128, 1)),
            "bv": np.ascontiguousarray(bv[sl].reshape(128, 1)),
            "bo": bo_arr,
            "tri": tri_arr,
            "identf": id_arr,
        }
        if mode == "generic":
            m["mtiles"] = _bf16(mt)
        in_maps.append(m)

    if trace:
        trace = _install_ntff_hook()
    res = bass_utils.run_bass_kernel_spmd(
        nc, in_maps, core_ids=list(range(NCORES)), trace=trace
    )
    # core k's y rows [128c : 128c+128] hold tokens [1024c + 128k, +128)
    out = np.empty((TT, D_MODEL), dtype=np.float32)
    for k in range(NCORES):
        yk = res.results[k]["y"]
        for c in range(NCH):
            out[1024 * c + 128 * k : 1024 * c + 128 * (k + 1)] = yk[
                128 * c : 128 * (c + 1)
            ]
    if trace:
        kernel._last_result = res
    return out.reshape(B, T, D_MODEL)


# revision 14
# speedup vs baseline: 1.3062x; 1.3062x over previous
"""Causal self-attention Trainium2 Bass kernel.

Sharding: 8-way head tensor-parallelism for QKV projections + attention
(2 heads per core, full batch).  The output projection is re-sharded via
FOUR chunked AllToAlls (one per pair of 512-token blocks, 128-token
stripes per core) so the exchange overlaps attention compute; each core
then projects 4x128 = 512 token rows and the host re-interleaves.

Fused software pipeline per 512-token block g (b = g//4, a = g%4):
  norm(g-1) [ACT ln -> exp(-x) gives 1/den, K=1 matmul broadcast,
  DVE multiply] -> collective chunk (every 2nd block) -> QK projection
  for block g -> output projection of the previous chunk -> V
  projection + transpose -> attention jc-loop (S^T = K^T.T @ Q^T,
  exp on ACT, PV accumulate; PV emission delayed one jc so the PE
  never waits on the ACT exp).

This keeps the PE array continuously busy (avoids the 2x mid-p-state
clock penalty) and leaves only the last collective + 128-row output
projection exposed at the end.

All matmul operands are bf16 (fp32 PSUM accumulation); measured
end-to-end relative error vs the fp32 reference is ~3e-3.

PSUM budget (8 banks): pool A [128,2,512]f32 x2 bufs (4 banks) shared
by QK-proj / V+transposes / ST tiles / rcp-broadcast / yproj, and pool
pv [65,2,512]f32 x2 bufs (4 banks) for the PV accumulators.
"""

import math
import os

import numpy as np

os.environ.setdefault("JAX_COMPILATION_CACHE_DIR", "/tmp/jax_cache")

D_MODEL = 1024
NUM_HEADS = 16
D_K = 64
B = 2
T = 2048
TT = B * T          # 4096 flattened tokens
NCORES = 8
HL = NUM_HEADS // NCORES   # heads per core = 2
DO = D_MODEL // 128        # 8 contraction chunks
NB = TT // 512             # 8 blocks (b, a)
NI = T // 512              # 4 query chunks per batch
NJ = T // 128              # 16 key chunks per batch
NCH = NB // 2              # 4 AllToAll chunks (2 blocks each)
SH = TT // NCORES          # 512 output rows per core

_cache = {}


def _install_ntff_hook():
    """The agent image's antenv lacks axon_hooks; replicate what
    trn_agent_boot would register so trace=True can capture NTFFs."""
    import sys
    import types

    try:
        from antenv import axon_hooks  # noqa: F401
        return True
    except ImportError:
        pass
    try:
        import antenv
        from trn_agent_boot.trn_boot import _ntff_profile_via_ctypes

        mod = types.ModuleType("antenv.axon_hooks")
        holder = [None]
        mod.set_axon_ntff_profile_hook = lambda h: holder.__setitem__(0, h)
        mod.get_axon_ntff_profile_hook = lambda: holder[0]
        sys.modules["antenv.axon_hooks"] = mod
        antenv.axon_hooks = mod
        mod.set_axon_ntff_profile_hook(
            _ntff_profile_via_ctypes("/opt/axon/libaxon_pjrt.so")
        )
        return True
    except Exception:
        return False


def _build_module(mode, blocks=None, n_mtiles=1):
    """Build + compile the Bass module.

    mode: "causal" (tril mask), "ones" (no masking), "generic"
    blocks: for generic mode, blocks[jc][a] = 0 skip / 1 full / (2, idx) mixed
    """
    from contextlib import ExitStack

    import concourse.mybir as mybir
    import concourse.tile as tile
    from concourse import bacc

    F32 = mybir.dt.float32
    BF16 = mybir.dt.bfloat16
    AF = mybir.ActivationFunctionType

    nc = bacc.Bacc(
        "TRN2",
        target_bir_lowering=False,
        debug=False,
        enable_asserts=False,
        num_devices=NCORES,
    )

    xT = nc.dram_tensor("xT", [128, DO, TT], BF16, kind="ExternalInput").ap()
    wq = nc.dram_tensor("wq", [128, DO, 128], BF16, kind="ExternalInput").ap()
    wk = nc.dram_tensor("wk", [128, DO, 128], BF16, kind="ExternalInput").ap()
    wv = nc.dram_tensor("wv", [128, DO, 128], BF16, kind="ExternalInput").ap()
    wo = nc.dram_tensor("wo", [128, DO, 1024], BF16, kind="ExternalInput").ap()
    bqin = nc.dram_tensor("bq", [128, 1], F32, kind="ExternalInput").ap()
    bkin = nc.dram_tensor("bk", [128, 1], F32, kind="ExternalInput").ap()
    bvin = nc.dram_tensor("bv", [128, 1], F32, kind="ExternalInput").ap()
    boin = nc.dram_tensor("bo", [1, 1024], F32, kind="ExternalInput").ap()
    tri_in = nc.dram_tensor("tri", [128, 128], BF16, kind="ExternalInput").ap()
    id_in = nc.dram_tensor("identf", [128, 128], F32, kind="ExternalInput").ap()
    if mode == "generic":
        mtiles = nc.dram_tensor(
            "mtiles", [n_mtiles, 128, 512], BF16, kind="ExternalInput"
        ).ap()
    y = nc.dram_tensor("y", [SH, 1024], F32, kind="ExternalOutput").ap()

    def jcs_of(b, a):
        if mode == "causal":
            return list(range(4 * a + 4))
        if mode == "ones":
            return list(range(NJ))
        return [jc for jc in range(NJ) if blocks[jc][a] != 0]

    with tile.TileContext(nc) as tc, ExitStack() as ctx:
        pers = ctx.enter_context(tc.tile_pool(name="pers", bufs=1))
        # PSUM: pool A (QK proj, V+transposes, ST pairs, rcp bcast, yproj)
        # 2 bufs x [128,2,512] f32 = 4 banks; pool pv 2 bufs = 4 banks.
        pA = ctx.enter_context(tc.tile_pool(name="pA", bufs=2, space="PSUM"))
        pV = ctx.enter_context(tc.tile_pool(name="pV", bufs=2, space="PSUM"))

        # ---- persistent SBUF (weights etc.; wo deferred to block 1) ----
        wq_sb = pers.tile([128, DO, 128], BF16, name="wq_sb")
        nc.sync.dma_start(wq_sb[:], wq[:])
        wk_sb = pers.tile([128, DO, 128], BF16, name="wk_sb")
        nc.sync.dma_start(wk_sb[:], wk[:])
        wv_sb = pers.tile([128, DO, 128], BF16, name="wv_sb")
        nc.sync.dma_start(wv_sb[:], wv[:])
        bq_sb = pers.tile([128, 1], F32, name="bq_sb")
        nc.sync.dma_start(bq_sb[:], bqin[:])
        bk_sb = pers.tile([128, 1], F32, name="bk_sb")
        nc.sync.dma_start(bk_sb[:], bkin[:])
        bv_sb = pers.tile([128, 1], F32, name="bv_sb")
        nc.sync.dma_start(bv_sb[:], bvin[:])
        bo_sb = pers.tile([1, 1024], F32, name="bo_sb")
        nc.sync.dma_start(bo_sb[:], boin[:])
        tri_full = pers.tile([128, 128], BF16, name="tri_full")
        nc.sync.dma_start(tri_full[:], tri_in[:])
        tri_sb = tri_full[:, 0:128]
        ident_t = pers.tile([128, 128], F32, name="ident_t")
        nc.sync.dma_start(ident_t[:], id_in[:])
        ident = ident_t[:]
        wo_sb = pers.tile([128, DO, 1024], BF16, name="wo_sb")

        ones_bf = pers.tile([128, 128], BF16, name="ones_bf")
        nc.vector.memset(ones_bf[:], 1.0)
        ones_f32 = pers.tile([1, 128], F32, name="ones_f32")
        nc.vector.memset(ones_f32[:], 1.0)

        qt = pers.tile([128, TT], BF16, name="qt")
        kt = pers.tile([128, TT], BF16, name="kt")
        vsb = pers.tile([128, B * NJ, HL, 80], BF16, name="vsb")
        nc.vector.tensor_copy(
            vsb[:, :, :, 64],
            ones_bf[:, 0 : B * NJ * HL].rearrange("p (a b) -> p a b", a=B * NJ),
        )

        # broadcast bo across partitions once (fp32, one-time)
        bob = pers.tile([128, 1024], F32, name="bob")
        bps = pA.tile([128, 2, 512], F32, name="bps", tag="A")
        for i in range(2):
            nc.tensor.matmul(
                bps[:, i, :],
                ones_f32[:, :],
                bo_sb[:, 512 * i : 512 * (i + 1)],
                start=True,
                stop=True,
            )
        nc.vector.tensor_copy(bob[:], bps.rearrange("p a b -> p (a b)"))

        # DRAM staging for the 4 chunked AllToAlls
        dramp = ctx.enter_context(tc.tile_pool(name="dramp", bufs=1, space="DRAM"))
        a2a_in = [
            dramp.tile([NCORES, 128, 128], BF16, name=f"a2a_in{c}")
            for c in range(NCH)
        ]
        a2a_out = [
            dramp.tile([NCORES, 128, 128], BF16, name=f"a2a_out{c}")
            for c in range(NCH)
        ]

        xtp = ctx.enter_context(tc.tile_pool(name="xtp", bufs=2))
        vtp = ctx.enter_context(tc.tile_pool(name="vtp", bufs=2))
        sxp = ctx.enter_context(tc.tile_pool(name="sxp", bufs=4))
        normp = ctx.enter_context(tc.tile_pool(name="normp", bufs=2))
        obp = ctx.enter_context(tc.tile_pool(name="obp", bufs=2))
        oap = ctx.enter_context(tc.tile_pool(name="oap", bufs=2))
        yp = ctx.enter_context(tc.tile_pool(name="yp", bufs=2))
        mtp = ctx.enter_context(tc.tile_pool(name="mtp", bufs=2))

        xt_tiles = [None] * NB

        def prefetch_x(g):
            xt_tiles[g] = xtp.tile([128, DO, 512], BF16, name=f"xt{g}", tag="xt")
            nc.sync.dma_start(xt_tiles[g][:], xT[:, :, 512 * g : 512 * (g + 1)])

        def emit_norm1(g, pv):
            """Stage 1 (emitted one block after g): reciprocal of the
            softmax denominators on DVE (custom approx op, no ACT tables)
            and drain pv from PSUM to SBUF via ACT Copy (table-free)."""
            if pv is None:  # generic mode, fully-masked block
                return None
            dsb = normp.tile([128, HL, 512], F32, name=f"dsb{g}", tag="dsb")
            nc.scalar.activation(dsb[64:65, :, :], pv[64:65, :, :], AF.Copy)
            rcf = normp.tile([128, HL, 512], F32, name=f"rcf{g}", tag="rcf")
            nc.vector.reciprocal(rcf[64:65, :, :], dsb[64:65, :, :])
            rcpr = normp.tile([128, HL, 512], BF16, name=f"rcp{g}", tag="rcp")
            nc.scalar.activation(rcpr[64:65, :, :], rcf[64:65, :, :], AF.Copy)
            pvc = normp.tile([128, HL, 512], BF16, name=f"pvc{g}", tag="pvc")
            nc.scalar.activation(pvc[0:64, :, :], pv[0:64, :, :], AF.Copy)
            return (rcpr, pvc)

        def emit_norm2(g, st1):
            """Stage 2 (two blocks after g): K=1 matmul broadcast of the
            reciprocal rows, DVE multiply, stage into the a2a input."""
            if st1 is None:
                ob = obp.tile([64, HL, 512], BF16, name=f"ob{g}", tag="ob")
                nc.vector.memset(ob[:], 0.0)
            else:
                rcpr, pvc = st1
                rbt = pA.tile([128, 2, 512], F32, name=f"rb{g}", tag="A")
                for h in range(HL):
                    nc.tensor.matmul(
                        rbt[0:64, h, :],
                        ones_bf[64:65, 0:64],
                        rcpr[64:65, h, :],
                        start=True,
                        stop=True,
                        tile_position=(64, 0),
                    )
                ob = obp.tile([64, HL, 512], BF16, name=f"ob{g}", tag="ob")
                nc.vector.tensor_mul(ob[:], pvc[0:64, :, :], rbt[0:64, :, :])
            # stage into the a2a input: chunk c = g//2; block supplies
            # destination stripes 4*(g%2) .. 4*(g%2)+3 (128 tokens each).
            c, q = g // 2, g % 2
            for d4 in range(4):
                d = 4 * q + d4
                for h in range(HL):
                    nc.sync.dma_start(
                        a2a_in[c][d, 64 * h : 64 * (h + 1), :],
                        ob[:, h, 128 * d4 : 128 * (d4 + 1)],
                    )

        def emit_collective(c):
            nc.gpsimd.collective_compute(
                "AllToAll",
                mybir.AluOpType.bypass,
                replica_groups=[list(range(NCORES))],
                ins=[a2a_in[c].opt()],
                outs=[a2a_out[c].opt()],
            )

        def emit_yproj(c):
            """Output projection for this core's 128 tokens of chunk c."""
            oa = oap.tile([128, NCORES, 128], BF16, name=f"oa{c}", tag="oa")
            nc.sync.dma_start(oa[:], a2a_out[c].rearrange("r p t -> p r t"))
            yps = pA.tile([128, 2, 512], F32, name=f"yps{c}", tag="A")
            for oc in range(2):
                for do in range(DO):
                    nc.tensor.matmul(
                        yps[:, oc, :],
                        oa[:, do, :],
                        wo_sb[:, do, 512 * oc : 512 * (oc + 1)],
                        start=(do == 0),
                        stop=(do == DO - 1),
                    )
            yt = yp.tile([128, 1024], F32, name=f"yt{c}", tag="y")
            nc.vector.tensor_add(yt[:], yps.rearrange("p a b -> p (a b)"), bob[:])
            nc.sync.dma_start(y[128 * c : 128 * (c + 1), :], yt[:])

        prefetch_x(0)
        pend1 = None  # (g, pv) awaiting norm stage 1 (one block back)
        pend2 = None  # (g, st1) awaiting norm stage 2 (two blocks back)
        for g in range(NB):
            b, a = g // 4, g % 4
            ii0 = 512 * g
            if g + 1 < NB:
                prefetch_x(g + 1)

            # ---- stage 2 of the block-(g-2) normalization ----
            if pend2 is not None:
                emit_norm2(*pend2)
                pend2 = None
            # chunk c covers blocks (2c, 2c+1); its staging completes in
            # stage2(2c+1) above, i.e. when g == 2c+3
            if g >= 3 and g % 2 == 1:
                emit_collective((g - 3) // 2)

            # ---- QK projection for block g ----
            xt_t = xt_tiles[g]
            qk = pA.tile([128, 2, 512], F32, name=f"qk{g}", tag="A")
            for col, w_sb in ((0, wq_sb), (1, wk_sb)):
                for do in range(DO):
                    nc.tensor.matmul(
                        qk[:, col, :],
                        w_sb[:, do, :],
                        xt_t[:, do, :],
                        start=(do == 0),
                        stop=(do == DO - 1),
                    )
            nc.vector.tensor_scalar_add(
                qt[:, ii0 : ii0 + 512], qk[:, 0, :], bq_sb[:]
            )
            nc.vector.tensor_scalar_add(
                kt[:, ii0 : ii0 + 512], qk[:, 1, :], bk_sb[:]
            )

            # ---- stage 1 of the block-(g-1) normalization ----
            if pend1 is not None:
                pg, ppv = pend1
                pend2 = (pg, emit_norm1(pg, ppv))
                pend1 = None

            if g == 1:
                nc.sync.dma_start(wo_sb[:], wo[:])
            # yproj(c) one block after collective c to absorb skew
            if g >= 4 and g % 2 == 0:
                emit_yproj((g - 4) // 2)

            # ---- V projection + transpose for block g ----
            vtr = pA.tile([128, 2, 512], F32, name=f"vtr{g}", tag="A")
            for do in range(DO):
                nc.tensor.matmul(
                    vtr[:, 0, :],
                    wv_sb[:, do, :],
                    xt_t[:, do, :],
                    start=(do == 0),
                    stop=(do == DO - 1),
                )
            vt_t = vtp.tile([128, 512], F32, name=f"vt{g}", tag="vt")
            nc.vector.tensor_scalar_add(vt_t[:], vtr[:, 0, :], bv_sb[:])
            tps = vtr[:, 1, :].rearrange("p (k t) -> p k t", k=4)
            for k in range(4):
                nc.tensor.transpose(
                    tps[:, k, :], vt_t[:, 128 * k : 128 * (k + 1)], ident
                )
            nc.vector.tensor_copy(
                vsb[:, 4 * g : 4 * (g + 1), :, 0:64],
                tps.rearrange("p k (h c) -> p k h c", h=HL),
            )

            # ---- attention for block g ----
            jcs = jcs_of(b, a)
            if not jcs:
                pend1 = (g, None)
                continue
            pv = pV.tile([65, 2, 512], F32, name=f"pv{g}", tag="pv")
            pvs = [pv[:, h, :] for h in range(HL)]
            prev = None  # (jc, s, w, first) awaiting PV emission
            for idx, jc in enumerate(jcs):
                j0 = b * T + 128 * jc
                diag = mode == "causal" and jc >= 4 * a
                s = 128 * (jc - 4 * a) if diag else 0
                w = 512 - s
                st = pA.tile([128, 2, 512], F32, name=f"st{g}_{jc}", tag="A")
                for h in range(HL):
                    nc.tensor.matmul(
                        st[:, h, 0:w],
                        kt[64 * h : 64 * (h + 1), j0 : j0 + 128],
                        qt[64 * h : 64 * (h + 1), ii0 + s : ii0 + 512],
                        start=True,
                        stop=True,
                        tile_position=(64 * h, 0),
                    )
                ex = sxp.tile([128, 2, 512], BF16, name=f"ex{g}_{jc}", tag="ex")
                nc.scalar.activation(ex[:, :, 0:w], st[:, :, 0:w], AF.Exp)
                if diag:
                    for h in range(HL):
                        nc.vector.tensor_mul(
                            ex[:, h, 0:128], ex[:, h, 0:128], tri_sb
                        )
                if mode == "generic" and blocks[jc][a] != 1:
                    mt = mtp.tile([128, 512], BF16, name=f"mt{g}_{jc}", tag="mt")
                    nc.sync.dma_start(mt[:], mtiles[blocks[jc][a][1]])
                    for h in range(HL):
                        nc.vector.tensor_mul(ex[:, h, :], ex[:, h, :], mt[:])
                # delayed PV emission: the PE never stalls on this jc's exp
                if prev is not None:
                    pjc, ps, pw, pfirst, pex = prev
                    for h in range(HL):
                        nc.tensor.matmul(
                            pvs[h][:, ps : ps + pw],
                            vsb[:, b * NJ + pjc, h, 0:65],
                            pex[:, h, 0:pw],
                            start=pfirst,
                            stop=False,
                        )
                prev = (jc, s, w, idx == 0, ex)
            pjc, ps, pw, pfirst, pex = prev
            for h in range(HL):
                nc.tensor.matmul(
                    pvs[h][:, ps : ps + pw],
                    vsb[:, b * NJ + pjc, h, 0:65],
                    pex[:, h, 0:pw],
                    start=pfirst,
                    stop=True,
                )
            pend1 = (g, pv)

        # ---- tail: finish blocks 6,7 norms; last collective; yproj 2,3 ----
        if pend2 is not None:
            emit_norm2(*pend2)
        pg, ppv = pend1
        emit_norm2(pg, emit_norm1(pg, ppv))
        emit_collective(NCH - 1)
        emit_yproj(NCH - 2)
        emit_yproj(NCH - 1)

    nc.compile()
    return nc


def _detect_mode(mask):
    m2 = np.asarray(mask).reshape(T, T)
    if np.array_equal(m2, np.tril(np.ones((T, T), m2.dtype))):
        return "causal", None, None
    if np.all(m2 != 0):
        return "ones", None, None
    # generic: classify [jc, a] blocks of mask^T
    mT = (m2 != 0).T.astype(np.float32)  # [j, i]
    blocks = [[0] * NI for _ in range(NJ)]
    tiles = []
    seen = {}
    for jc in range(NJ):
        for a in range(NI):
            sub = mT[128 * jc : 128 * (jc + 1), 512 * a : 512 * (a + 1)]
            if not sub.any():
                blocks[jc][a] = 0
            elif sub.all():
                blocks[jc][a] = 1
            else:
                key = sub.tobytes()
                if key not in seen:
                    seen[key] = len(tiles)
                    tiles.append(sub.copy())
                blocks[jc][a] = (2, seen[key])
    mt = np.stack(tiles) if tiles else np.zeros((1, 128, 512), np.float32)
    return "generic", blocks, mt


def _bf16(a):
    import ml_dtypes

    return np.ascontiguousarray(a, dtype=np.float32).astype(ml_dtypes.bfloat16)


def _rearr_w(w):
    # [D, M] -> [128, DO, M] as (d_inner, d_outer, m), bf16
    m = w.shape[1]
    return _bf16(
        np.ascontiguousarray(w, dtype=np.float32)
        .reshape(DO, 128, m)
        .transpose(1, 0, 2)
    )


def kernel(x, mask, Wq, bq, Wk, bk, Wv, bv, Wo, bo, trace=False):
    from concourse import bass_utils

    x = np.asarray(x, dtype=np.float32)
    Wq = np.asarray(Wq, dtype=np.float32)
    Wk = np.asarray(Wk, dtype=np.float32)
    Wv = np.asarray(Wv, dtype=np.float32)
    Wo = np.asarray(Wo, dtype=np.float32)
    bq = np.asarray(bq, dtype=np.float32)
    bk = np.asarray(bk, dtype=np.float32)
    bv = np.asarray(bv, dtype=np.float32)
    bo = np.asarray(bo, dtype=np.float32)

    mode, blocks, mt = _detect_mode(mask)
    cache_key = (mode, None if blocks is None else str(blocks))
    if cache_key not in _cache:
        _cache[cache_key] = _build_module(
            mode, blocks, 1 if mt is None else mt.shape[0]
        )
    nc = _cache[cache_key]

    scale = 1.0 / math.sqrt(D_K)
    xT_arr = _bf16(x.reshape(TT, D_MODEL).T.reshape(DO, 128, TT).transpose(1, 0, 2))
    wo_arr = _rearr_w(Wo)
    bo_arr = np.ascontiguousarray(bo.reshape(1, 1024))
    tri_arr = _bf16(np.triu(np.ones((128, 128), np.float32)))
    id_arr = np.eye(128, dtype=np.float32)

    in_maps = []
    for c in range(NCORES):
        sl = slice(128 * c, 128 * (c + 1))
        m = {
            "xT": xT_arr,
            "wq": _rearr_w(Wq[:, sl] * scale),
            "wk": _rearr_w(Wk[:, sl]),
            "wv": _rearr_w(Wv[:, sl]),
            "wo": wo_arr,
            "bq": np.ascontiguousarray((bq[sl] * scale).reshape(128, 1)),
            "bk": np.ascontiguousarray(bk[sl].reshape(128, 1)),
            "bv": np.ascontiguousarray(bv[sl].reshape(128, 1)),
            "bo": bo_arr,
            "tri": tri_arr,
            "identf": id_arr,
        }
        if mode == "generic":
            m["mtiles"] = _bf16(mt)
        in_maps.append(m)

    if trace:
        trace = _install_ntff_hook()
    res = bass_utils.run_bass_kernel_spmd(
        nc, in_maps, core_ids=list(range(NCORES)), trace=trace
    )
    # core k's y rows [128c : 128c+128] hold tokens [1024c + 128k, +128)
    out = np.empty((TT, D_MODEL), dtype=np.float32)
    for k in range(NCORES):
        yk = res.results[k]["y"]
        for c in range(NCH):
            out[1024 * c + 128 * k : 1024 * c + 128 * (k + 1)] = yk[
                128 * c : 128 * (c + 1)
            ]
    if trace:
        kernel._last_result = res
    return out.reshape(B, T, D_MODEL)


# revision 23
# speedup vs baseline: 1.3991x; 1.0711x over previous
"""Causal self-attention Trainium2 Bass kernel.

Sharding: 8-way head tensor-parallelism for QKV projections + attention
(2 heads per core, full batch).  The output projection is re-sharded via
FOUR chunked AllToAlls (one per pair of 512-token blocks, 128-token
stripes per core) so the exchange overlaps attention compute; each core
then projects 4x128 = 512 token rows and the host re-interleaves.

Fused software pipeline per 512-token block g (b = g//4, a = g%4):
  norm(g-1) [ACT ln -> exp(-x) gives 1/den, K=1 matmul broadcast,
  DVE multiply] -> collective chunk (every 2nd block) -> QK projection
  for block g -> output projection of the previous chunk -> V
  projection + transpose -> attention jc-loop (S^T = K^T.T @ Q^T,
  exp on ACT, PV accumulate; PV emission delayed one jc so the PE
  never waits on the ACT exp).

This keeps the PE array continuously busy (avoids the 2x mid-p-state
clock penalty) and leaves only the last collective + 128-row output
projection exposed at the end.

All matmul operands are bf16 (fp32 PSUM accumulation); measured
end-to-end relative error vs the fp32 reference is ~3e-3.

PSUM budget (8 banks): pool A [128,2,512]f32 x2 bufs (4 banks) shared
by QK-proj / V+transposes / ST tiles / rcp-broadcast / yproj, and pool
pv [65,2,512]f32 x2 bufs (4 banks) for the PV accumulators.
"""

import math
import os

import numpy as np

os.environ.setdefault("JAX_COMPILATION_CACHE_DIR", "/tmp/jax_cache")

D_MODEL = 1024
NUM_HEADS = 16
D_K = 64
B = 2
T = 2048
TT = B * T          # 4096 flattened tokens
NCORES = 8
HL = NUM_HEADS // NCORES   # heads per core = 2
DO = D_MODEL // 128        # 8 contraction chunks
NB = TT // 512             # 8 blocks (b, a)
NI = T // 512              # 4 query chunks per batch
NJ = T // 128              # 16 key chunks per batch
NCH = NB // 2              # 4 AllToAll chunks (2 blocks each)
SH = TT // NCORES          # 512 output rows per core

_cache = {}


def _install_ntff_hook():
    """The agent image's antenv lacks axon_hooks; replicate what
    trn_agent_boot would register so trace=True can capture NTFFs."""
    import sys
    import types

    try:
        from antenv import axon_hooks  # noqa: F401
        return True
    except ImportError:
        pass
    try:
        import antenv
        from trn_agent_boot.trn_boot import _ntff_profile_via_ctypes

        mod = types.ModuleType("antenv.axon_hooks")
        holder = [None]
        mod.set_axon_ntff_profile_hook = lambda h: holder.__setitem__(0, h)
        mod.get_axon_ntff_profile_hook = lambda: holder[0]
        sys.modules["antenv.axon_hooks"] = mod
        antenv.axon_hooks = mod
        mod.set_axon_ntff_profile_hook(
            _ntff_profile_via_ctypes("/opt/axon/libaxon_pjrt.so")
        )
        return True
    except Exception:
        return False


def _build_module(mode, blocks=None, n_mtiles=1):
    """Build + compile the Bass module.

    mode: "causal" (tril mask), "ones" (no masking), "generic"
    blocks: for generic mode, blocks[jc][a] = 0 skip / 1 full / (2, idx) mixed
    """
    from contextlib import ExitStack

    import concourse.mybir as mybir
    import concourse.tile as tile
    from concourse import bacc

    F32 = mybir.dt.float32
    BF16 = mybir.dt.bfloat16
    AF = mybir.ActivationFunctionType

    nc = bacc.Bacc(
        "TRN2",
        target_bir_lowering=False,
        debug=False,
        enable_asserts=False,
        num_devices=NCORES,
    )

    xT = nc.dram_tensor("xT", [128, DO, TT], BF16, kind="ExternalInput").ap()
    wq = nc.dram_tensor("wq", [128, DO, 128], BF16, kind="ExternalInput").ap()
    wk = nc.dram_tensor("wk", [128, DO, 128], BF16, kind="ExternalInput").ap()
    wv = nc.dram_tensor("wv", [128, DO, 128], BF16, kind="ExternalInput").ap()
    wo = nc.dram_tensor("wo", [128, DO, 1024], BF16, kind="ExternalInput").ap()
    bqin = nc.dram_tensor("bq", [128, 1], F32, kind="ExternalInput").ap()
    bkin = nc.dram_tensor("bk", [128, 1], F32, kind="ExternalInput").ap()
    bvin = nc.dram_tensor("bv", [128, 1], F32, kind="ExternalInput").ap()
    boin = nc.dram_tensor("bo", [1, 1024], F32, kind="ExternalInput").ap()
    tri_in = nc.dram_tensor("tri", [128, 128], BF16, kind="ExternalInput").ap()
    id_in = nc.dram_tensor("identf", [128, 128], F32, kind="ExternalInput").ap()
    if mode == "generic":
        mtiles = nc.dram_tensor(
            "mtiles", [n_mtiles, 128, 512], BF16, kind="ExternalInput"
        ).ap()
    y = nc.dram_tensor("y", [SH, 1024], F32, kind="ExternalOutput").ap()

    def jcs_of(b, a):
        if mode == "causal":
            return list(range(4 * a + 4))
        if mode == "ones":
            return list(range(NJ))
        return [jc for jc in range(NJ) if blocks[jc][a] != 0]

    with tile.TileContext(nc) as tc, ExitStack() as ctx:
        pers = ctx.enter_context(tc.tile_pool(name="pers", bufs=1))
        # PSUM: pool A (QK proj, V+transposes, ST pairs, rcp bcast, yproj)
        # 2 bufs x [128,2,512] f32 = 4 banks; pool pv 2 bufs = 4 banks.
        pA = ctx.enter_context(tc.tile_pool(name="pA", bufs=2, space="PSUM"))
        pV = ctx.enter_context(tc.tile_pool(name="pV", bufs=2, space="PSUM"))

        # ---- persistent SBUF (weights etc.; wo deferred to block 1) ----
        wq_sb = pers.tile([128, DO, 128], BF16, name="wq_sb")
        wk_sb = pers.tile([128, DO, 128], BF16, name="wk_sb")
        wv_sb = pers.tile([128, DO, 128], BF16, name="wv_sb")
        bq_sb = pers.tile([128, 1], F32, name="bq_sb")
        bk_sb = pers.tile([128, 1], F32, name="bk_sb")
        bv_sb = pers.tile([128, 1], F32, name="bv_sb")
        bo_sb = pers.tile([1, 1024], F32, name="bo_sb")
        tri_full = pers.tile([128, 128], BF16, name="tri_full")
        tri_sb = tri_full[:, 0:128]
        ident_t = pers.tile([128, 128], F32, name="ident_t")
        ident = ident_t[:]
        wo_sb = pers.tile([128, DO, 1024], BF16, name="wo_sb")

        ones_bf = pers.tile([128, 128], BF16, name="ones_bf")
        nc.vector.memset(ones_bf[:], 1.0)
        ones_f32 = pers.tile([1, 128], F32, name="ones_f32")
        nc.vector.memset(ones_f32[:], 1.0)

        qt = pers.tile([128, TT], BF16, name="qt")
        kt = pers.tile([128, TT], BF16, name="kt")
        vsb = pers.tile([128, B * NJ, HL, 80], BF16, name="vsb")
        nc.vector.tensor_copy(
            vsb[:, :, :, 64],
            ones_bf[:, 0 : B * NJ * HL].rearrange("p (a b) -> p a b", a=B * NJ),
        )

        bob = pers.tile([128, 1024], F32, name="bob")

        # DRAM staging for the 4 chunked AllToAlls
        dramp = ctx.enter_context(tc.tile_pool(name="dramp", bufs=1, space="DRAM"))
        a2a_in = [
            dramp.tile([NCORES, 128, 128], BF16, name=f"a2a_in{c}")
            for c in range(NCH)
        ]
        a2a_out = [
            dramp.tile([NCORES, 128, 128], BF16, name=f"a2a_out{c}")
            for c in range(NCH)
        ]

        xtp = ctx.enter_context(tc.tile_pool(name="xtp", bufs=2))
        vtp = ctx.enter_context(tc.tile_pool(name="vtp", bufs=2))
        sxp = ctx.enter_context(tc.tile_pool(name="sxp", bufs=4))
        normp = ctx.enter_context(tc.tile_pool(name="normp", bufs=2))
        obp = ctx.enter_context(tc.tile_pool(name="obp", bufs=2))
        oap = ctx.enter_context(tc.tile_pool(name="oap", bufs=2))
        yp = ctx.enter_context(tc.tile_pool(name="yp", bufs=2))
        mtp = ctx.enter_context(tc.tile_pool(name="mtp", bufs=2))

        xt_tiles = [None] * NB

        def prefetch_x(g):
            xt_tiles[g] = xtp.tile([128, DO, 512], BF16, name=f"xt{g}", tag="xt")
            nc.sync.dma_start(xt_tiles[g][:], xT[:, :, 512 * g : 512 * (g + 1)])

        # x block 0 first so the first projection isn't stuck behind the
        # weight loads in the DMA queues; then the weights.
        prefetch_x(0)
        nc.sync.dma_start(wq_sb[:], wq[:])
        nc.sync.dma_start(wk_sb[:], wk[:])
        nc.sync.dma_start(bq_sb[:], bqin[:])
        nc.sync.dma_start(bk_sb[:], bkin[:])
        nc.sync.dma_start(wv_sb[:], wv[:])
        nc.sync.dma_start(bv_sb[:], bvin[:])
        nc.sync.dma_start(tri_full[:], tri_in[:])
        nc.sync.dma_start(ident_t[:], id_in[:])
        nc.sync.dma_start(bo_sb[:], boin[:])

        def emit_norm1(g, pv):
            """Stage 1 (emitted one block after g): reciprocal of the
            softmax denominators on DVE (custom approx op, no ACT tables)
            and drain pv from PSUM to SBUF via ACT Copy (table-free)."""
            if pv is None:  # generic mode, fully-masked block
                return None
            dsb = normp.tile([128, HL, 512], F32, name=f"dsb{g}", tag="dsb")
            nc.scalar.activation(dsb[64:65, :, :], pv[64:65, :, :], AF.Copy)
            rcf = normp.tile([128, HL, 512], F32, name=f"rcf{g}", tag="rcf")
            nc.vector.reciprocal(rcf[64:65, :, :], dsb[64:65, :, :])
            rcpr = normp.tile([128, HL, 512], BF16, name=f"rcp{g}", tag="rcp")
            nc.scalar.activation(rcpr[64:65, :, :], rcf[64:65, :, :], AF.Copy)
            pvc = normp.tile([128, HL, 512], BF16, name=f"pvc{g}", tag="pvc")
            nc.scalar.activation(pvc[0:64, :, :], pv[0:64, :, :], AF.Copy)
            return (rcpr, pvc)

        def emit_norm2(g, st1):
            """Stage 2 (two blocks after g): K=1 matmul broadcast of the
            reciprocal rows, DVE multiply, stage into the a2a input."""
            if st1 is None:
                ob = obp.tile([64, HL, 512], BF16, name=f"ob{g}", tag="ob")
                nc.vector.memset(ob[:], 0.0)
            else:
                rcpr, pvc = st1
                rbt = pA.tile([128, 2, 512], F32, name=f"rb{g}", tag="A")
                for h in range(HL):
                    nc.tensor.matmul(
                        rbt[0:64, h, :],
                        ones_bf[64:65, 0:64],
                        rcpr[64:65, h, :],
                        start=True,
                        stop=True,
                        tile_position=(64, 0),
                    )
                ob = obp.tile([64, HL, 512], BF16, name=f"ob{g}", tag="ob")
                nc.vector.tensor_mul(ob[:], pvc[0:64, :, :], rbt[0:64, :, :])
            # stage into the a2a input: chunk c = g//2; block supplies
            # destination stripes 4*(g%2) .. 4*(g%2)+3 (128 tokens each).
            c, q = g // 2, g % 2
            for h in range(HL):
                nc.sync.dma_start(
                    a2a_in[c][4 * q : 4 * q + 4, 64 * h : 64 * (h + 1), :]
                    .rearrange("d p t -> p d t"),
                    ob[:, h, :].rearrange("p (d t) -> p d t", d=4),
                )

        def emit_collective(c):
            nc.gpsimd.collective_compute(
                "AllToAll",
                mybir.AluOpType.bypass,
                replica_groups=[list(range(NCORES))],
                ins=[a2a_in[c].opt()],
                outs=[a2a_out[c].opt()],
            )

        def emit_yproj(c):
            """Output projection for this core's 128 tokens of chunk c."""
            oa = oap.tile([128, NCORES, 128], BF16, name=f"oa{c}", tag="oa")
            nc.sync.dma_start(oa[:], a2a_out[c].rearrange("r p t -> p r t"))
            yps = pA.tile([128, 2, 512], F32, name=f"yps{c}", tag="A")
            for oc in range(2):
                for do in range(DO):
                    nc.tensor.matmul(
                        yps[:, oc, :],
                        oa[:, do, :],
                        wo_sb[:, do, 512 * oc : 512 * (oc + 1)],
                        start=(do == 0),
                        stop=(do == DO - 1),
                    )
            yt = yp.tile([128, 1024], F32, name=f"yt{c}", tag="y")
            nc.vector.tensor_add(yt[:], yps.rearrange("p a b -> p (a b)"), bob[:])
            nc.sync.dma_start(y[128 * c : 128 * (c + 1), :], yt[:])

        pend1 = None  # (g, pv) awaiting norm stage 1 (one block back)
        pend2 = None  # (g, st1) awaiting norm stage 2 (two blocks back)
        for g in range(NB):
            b, a = g // 4, g % 4
            ii0 = 512 * g
            if g + 1 < NB:
                prefetch_x(g + 1)

            # ---- stage 2 of the block-(g-2) normalization ----
            if pend2 is not None:
                emit_norm2(*pend2)
                pend2 = None
            # chunk c covers blocks (2c, 2c+1); its staging completes in
            # stage2(2c+1) above, i.e. when g == 2c+3
            if g >= 3 and g % 2 == 1:
                emit_collective((g - 3) // 2)

            # ---- QK projection for block g ----
            xt_t = xt_tiles[g]
            qk = pA.tile([128, 2, 512], F32, name=f"qk{g}", tag="A")
            for col, w_sb in ((0, wq_sb), (1, wk_sb)):
                for do in range(DO):
                    nc.tensor.matmul(
                        qk[:, col, :],
                        w_sb[:, do, :],
                        xt_t[:, do, :],
                        start=(do == 0),
                        stop=(do == DO - 1),
                    )
            nc.vector.tensor_scalar_add(
                qt[:, ii0 : ii0 + 512], qk[:, 0, :], bq_sb[:]
            )
            nc.vector.tensor_scalar_add(
                kt[:, ii0 : ii0 + 512], qk[:, 1, :], bk_sb[:]
            )

            # ---- stage 1 of the block-(g-1) normalization ----
            if pend1 is not None:
                pg, ppv = pend1
                pend2 = (pg, emit_norm1(pg, ppv))
                pend1 = None

            if g == 1:
                nc.sync.dma_start(wo_sb[:], wo[:])
            if g == 2:
                # broadcast bo across partitions (needed by yproj at g=4)
                bps = pA.tile([128, 2, 512], F32, name="bps", tag="A")
                for i in range(2):
                    nc.tensor.matmul(
                        bps[:, i, :],
                        ones_f32[:, :],
                        bo_sb[:, 512 * i : 512 * (i + 1)],
                        start=True,
                        stop=True,
                    )
                nc.vector.tensor_copy(bob[:], bps.rearrange("p a b -> p (a b)"))
            # yproj(c) one block after collective c to absorb skew
            if g >= 4 and g % 2 == 0:
                emit_yproj((g - 4) // 2)

            # ---- V projection + transpose for block g ----
            vtr = pA.tile([128, 2, 512], F32, name=f"vtr{g}", tag="A")
            for do in range(DO):
                nc.tensor.matmul(
                    vtr[:, 0, :],
                    wv_sb[:, do, :],
                    xt_t[:, do, :],
                    start=(do == 0),
                    stop=(do == DO - 1),
                )
            vt_t = vtp.tile([128, 512], F32, name=f"vt{g}", tag="vt")
            nc.vector.tensor_scalar_add(vt_t[:], vtr[:, 0, :], bv_sb[:])
            tps = vtr[:, 1, :].rearrange("p (k t) -> p k t", k=4)
            for k in range(4):
                nc.tensor.transpose(
                    tps[:, k, :], vt_t[:, 128 * k : 128 * (k + 1)], ident
                )
            nc.vector.tensor_copy(
                vsb[:, 4 * g : 4 * (g + 1), :, 0:64],
                tps.rearrange("p k (h c) -> p k h c", h=HL),
            )

            # ---- attention for block g ----
            jcs = jcs_of(b, a)
            if not jcs:
                pend1 = (g, None)
                continue
            pv = pV.tile([65, 2, 512], F32, name=f"pv{g}", tag="pv")
            pvs = [pv[:, h, :] for h in range(HL)]

            def emit_pv(ent, last):
                pjc, ps, pw, pfirst, pex = ent
                for h in range(HL):
                    nc.tensor.matmul(
                        pvs[h][:, ps : ps + pw],
                        vsb[:, b * NJ + pjc, h, 0:65],
                        pex[:, h, 0:pw],
                        start=pfirst,
                        stop=last,
                    )

            pvq = []  # PV emission delayed 2 jcs: the PE never waits on exp
            for idx, jc in enumerate(jcs):
                j0 = b * T + 128 * jc
                diag = mode == "causal" and jc >= 4 * a
                s = 128 * (jc - 4 * a) if diag else 0
                w = 512 - s
                st = pA.tile([128, 2, 512], F32, name=f"st{g}_{jc}", tag="A")
                for h in range(HL):
                    nc.tensor.matmul(
                        st[:, h, 0:w],
                        kt[64 * h : 64 * (h + 1), j0 : j0 + 128],
                        qt[64 * h : 64 * (h + 1), ii0 + s : ii0 + 512],
                        start=True,
                        stop=True,
                        tile_position=(64 * h, 0),
                    )
                ex = sxp.tile([128, 2, 512], BF16, name=f"ex{g}_{jc}", tag="ex")
                nc.scalar.activation(ex[:, :, 0:w], st[:, :, 0:w], AF.Exp)
                if diag:
                    for h in range(HL):
                        nc.vector.tensor_mul(
                            ex[:, h, 0:128], ex[:, h, 0:128], tri_sb
                        )
                if mode == "generic" and blocks[jc][a] != 1:
                    mt = mtp.tile([128, 512], BF16, name=f"mt{g}_{jc}", tag="mt")
                    nc.sync.dma_start(mt[:], mtiles[blocks[jc][a][1]])
                    for h in range(HL):
                        nc.vector.tensor_mul(ex[:, h, :], ex[:, h, :], mt[:])
                pvq.append((jc, s, w, idx == 0, ex))
                if len(pvq) > 2:
                    emit_pv(pvq.pop(0), False)
            while pvq:
                emit_pv(pvq.pop(0), not pvq)
            pend1 = (g, pv)

        # ---- tail: yproj(2) overlaps the final norms; last collective ----
        emit_yproj(NCH - 2)
        if pend2 is not None:
            emit_norm2(*pend2)
        pg, ppv = pend1
        emit_norm2(pg, emit_norm1(pg, ppv))
        emit_collective(NCH - 1)
        emit_yproj(NCH - 1)

    nc.compile()
    return nc


def _detect_mode(mask):
    m2 = np.asarray(mask).reshape(T, T)
    if np.array_equal(m2, np.tril(np.ones((T, T), m2.dtype))):
        return "causal", None, None
    if np.all(m2 != 0):
        return "ones", None, None
    # generic: classify [jc, a] blocks of mask^T
    mT = (m2 != 0).T.astype(np.float32)  # [j, i]
    blocks = [[0] * NI for _ in range(NJ)]
    tiles = []
    seen = {}
    for jc in range(NJ):
        for a in range(NI):
            sub = mT[128 * jc : 128 * (jc + 1), 512 * a : 512 * (a + 1)]
            if not sub.any():
                blocks[jc][a] = 0
            elif sub.all():
                blocks[jc][a] = 1
            else:
                key = sub.tobytes()
                if key not in seen:
                    seen[key] = len(tiles)
                    tiles.append(sub.copy())
                blocks[jc][a] = (2, seen[key])
    mt = np.stack(tiles) if tiles else np.zeros((1, 128, 512), np.float32)
    return "generic", blocks, mt


def _bf16(a):
    import ml_dtypes

    return np.ascontiguousarray(a, dtype=np.float32).astype(ml_dtypes.bfloat16)


def _rearr_w(w):
    # [D, M] -> [128, DO, M] as (d_inner, d_outer, m), bf16
    m = w.shape[1]
    return _bf16(
        np.ascontiguousarray(w, dtype=np.float32)
        .reshape(DO, 128, m)
        .transpose(1, 0, 2)
    )


def kernel(x, mask, Wq, bq, Wk, bk, Wv, bv, Wo, bo, trace=False):
    from concourse import bass_utils

    x = np.asarray(x, dtype=np.float32)
    Wq = np.asarray(Wq, dtype=np.float32)
    Wk = np.asarray(Wk, dtype=np.float32)
    Wv = np.asarray(Wv, dtype=np.float32)
    Wo = np.asarray(Wo, dtype=np.float32)
    bq = np.asarray(bq, dtype=np.float32)
    bk = np.asarray(bk, dtype=np.float32)
    bv = np.asarray(bv, dtype=np.float32)
    bo = np.asarray(bo, dtype=np.float32)

    mode, blocks, mt = _detect_mode(mask)
    cache_key = (mode, None if blocks is None else str(blocks))
    if cache_key not in _cache:
        _cache[cache_key] = _build_module(
            mode, blocks, 1 if mt is None else mt.shape[0]
        )
    nc = _cache[cache_key]

    scale = 1.0 / math.sqrt(D_K)
    xT_arr = _bf16(x.reshape(TT, D_MODEL).T.reshape(DO, 128, TT).transpose(1, 0, 2))
    wo_arr = _rearr_w(Wo)
    bo_arr = np.ascontiguousarray(bo.reshape(1, 1024))
    tri_arr = _bf16(np.triu(np.ones((128, 128), np.float32)))
    id_arr = np.eye(128, dtype=np.float32)

    in_maps = []
    for c in range(NCORES):
        sl = slice(128 * c, 128 * (c + 1))
        m = {
            "xT": xT_arr,
            "wq": _rearr_w(Wq[:, sl] * scale),
            "wk": _rearr_w(Wk[:, sl]),
            "wv": _rearr_w(Wv[:, sl]),
            "wo": wo_arr,
            "bq": np.ascontiguousarray((bq[sl] * scale).reshape(128, 1)),
            "bk": np.ascontiguousarray(bk[sl].reshape(128, 1)),
            "bv": np.ascontiguousarray(bv[sl].reshape(128, 1)),
            "bo": bo_arr,
            "tri": tri_arr,
            "identf": id_arr,
        }
        if mode == "generic":
            m["mtiles"] = _bf16(mt)
        in_maps.append(m)

    if trace:
        trace = _install_ntff_hook()
    res = bass_utils.run_bass_kernel_spmd(
        nc, in_maps, core_ids=list(range(NCORES)), trace=trace
    )
    # core k's y rows [128c : 128c+128] hold tokens [1024c + 128k, +128)
    out = np.empty((TT, D_MODEL), dtype=np.float32)
    for k in range(NCORES):
        yk = res.results[k]["y"]
        for c in range(NCH):
            out[1024 * c + 128 * k : 1024 * c + 128 * (k + 1)] = yk[
                128 * c : 128 * (c + 1)
            ]
    if trace:
        kernel._last_result = res
    return out.reshape(B, T, D_MODEL)
